# revision 10
# baseline (speedup 1.0000x reference)
"""Trainium2 Bass kernel: CausalCrossConditionalSelfAttention.

Sharding: 8 cores = (batch b in {0,1}) x (head-group g in {0..3}); each core
computes attention for 2 heads (128 channels) of one batch element, plus its
tensor-parallel slice of the output projection. The host sums the 4 partial
projections per batch and adds b_proj.

On-device layout is transposed (channels on partitions):
  qT/kT: [128 (2 heads x 64 d), L]; S^T chunks [k-tile 128, q 512] so softmax
  reduction happens via a ones-column appended to V in the P^T@V matmul.
Block-causal mask structure is applied as 0/1 multiplicative masks on exp(S),
with fully-masked (k-tile, q-chunk) pairs skipped entirely.
"""

import ml_dtypes
import numpy as np

import concourse.bass as bass
import concourse.mybir as mybir
import concourse.tile as tile
from concourse import bacc
from concourse.bass_utils import run_bass_kernel_spmd
from concourse.masks import make_identity

B = 2
T = 1024
NSEG = 16
C = 512
NH = 8
HD = 64
L = 3 * T + 4 * NSEG  # 3136
N_CORES = 8

F32 = mybir.dt.float32
F32R = mybir.dt.float32r
BF16 = mybir.dt.bfloat16
AF = mybir.ActivationFunctionType
ALU = mybir.AluOpType

CHUNKS = [(0, 512), (512, 512), (1024, 512), (1536, 512), (2048, 512),
          (2560, 512), (3072, 64)]
NKT = 25  # key tiles of 128 (kt 24 has only 64 rows: the 4N text keys)

# Visibility of key-block bb from query-block r, as "keep iff q - k >= D'".
# None = invisible. STRICT marks exclusive (j < i) relations.
DPRIME = [[0, None, None], [1024, 1, -1023], [2048, 1024, 1]]
STRICT = [[False, None, None], [False, True, True], [False, False, True]]


def _pairs(ci):
    """(kt, z, tri) per key-tile for query chunk ci.

    z = number of fully-masked leading 128-subtiles (compute starts at col
    z*128); tri in {None, 'causal', 'strict', 'text'} selects the fixup
    applied to exp(S) for the partially-masked subtile."""
    q0, W = CHUNKS[ci]
    if ci == 6:
        return [(kt, 0, None) for kt in range(NKT)]
    r = q0 // T
    out = []
    for bb in range(3):
        Dp = DPRIME[r][bb]
        if Dp is None:
            continue
        st = STRICT[r][bb]
        D = Dp - 1 if st else Dp
        for kt in range(8 * bb, 8 * bb + 8):
            k0 = kt * 128
            if (q0 + W - 1) - k0 < Dp:
                continue  # fully masked
            if q0 - (k0 + 127) >= Dp:
                out.append((kt, 0, None))  # fully kept
            else:
                o = (k0 + D - q0) // 128
                out.append((kt, o, 'strict' if st else 'causal'))
    if r >= 1:
        out.append((24, 0, 'text'))
    return out


def _emit(nc, tc, d):
    from contextlib import ExitStack

    def fr(ap):
        return ap.bitcast(F32R)

    es = ExitStack()
    with es:
        const = es.enter_context(tc.tile_pool(name="const", bufs=1))
        persist = es.enter_context(tc.tile_pool(name="persist", bufs=1))

        identity = const.tile([128, 128], F32, tag="ident", name="identity")
        make_identity(nc, identity)
        # Build 0/1 triangular masks in f32, then round-copy into f32r tiles
        # (memset/affine_select cannot write f32r directly).
        causal01f = const.tile([128, 128], F32, tag="causal01f", name="causal01f")
        strict01f = const.tile([128, 128], F32, tag="strict01f", name="strict01f")
        causal01 = const.tile([128, 128], F32R, tag="causal01", name="causal01")
        strict01 = const.tile([128, 128], F32R, tag="strict01", name="strict01")
        for m01f, m01, op in ((causal01f, causal01, ALU.is_ge),
                              (strict01f, strict01, ALU.is_gt)):
            nc.vector.memset(m01f, 1.0)
            # keep (value (-1)*p + 1*f >= / > 0), else fill 0
            nc.gpsimd.affine_select(out=m01f, in_=m01f, pattern=[[1, 128]],
                                    compare_op=op, fill=0.0, base=0,
                                    channel_multiplier=-1)
            nc.vector.tensor_copy(m01, m01f)

        ones_col = const.tile([128, 1], F32, tag="ones_col", name="ones_col")
        nc.vector.memset(ones_col, 1.0)
        ones1f = const.tile([1, 64], F32, tag="ones1f", name="ones1f")
        nc.vector.memset(ones1f, 1.0)
        ones1 = const.tile([1, 64], F32R, tag="ones1", name="ones1")
        nc.vector.tensor_copy(ones1, ones1f)

        wq_sb = const.tile([128, 512], BF16, tag="wq", name="wq_sb")
        wk_sb = const.tile([128, 512], BF16, tag="wk", name="wk_sb")
        wv_sb = const.tile([128, 512], BF16, tag="wv", name="wv_sb")
        for sb, nm in ((wq_sb, 'wqT'), (wk_sb, 'wkT'), (wv_sb, 'wvT')):
            for ct in range(4):
                nc.sync.dma_start(out=sb[:, ct * 128:(ct + 1) * 128],
                                  in_=d[nm][ct * 128:(ct + 1) * 128, :])
        wp_stage = const.tile([128, 512], F32, tag="wps", name="wp_stage")
        nc.sync.dma_start(out=wp_stage, in_=d['wpT'][:, :])
        wp_sb = const.tile([128, 512], F32R, tag="wp", name="wp_sb")
        nc.vector.tensor_copy(wp_sb, wp_stage)
        up01_st = const.tile([64, T], F32, tag="up01st", name="up01_st")
        low01_st = const.tile([64, T], F32, tag="low01st", name="low01_st")
        nc.sync.dma_start(out=up01_st, in_=d['up01'][:, :])
        nc.sync.dma_start(out=low01_st, in_=d['low01'][:, :])
        up01_sb = const.tile([64, T], F32R, tag="up01", name="up01_sb")
        low01_sb = const.tile([64, T], F32R, tag="low01", name="low01_sb")
        nc.vector.tensor_copy(up01_sb, up01_st)
        nc.vector.tensor_copy(low01_sb, low01_st)

        # Persistent per-chunk tensors
        qt_t, kt_t, yt_t = [], [], []
        for ci, (q0, W) in enumerate(CHUNKS):
            qt_t.append(persist.tile([128, W], F32R, tag=f"qt{ci}", name=f"qt{ci}"))
            kt_t.append(persist.tile([128, W], F32R, tag=f"kt{ci}", name=f"kt{ci}"))
            yt_t.append(persist.tile([128, W], F32R, tag=f"yt{ci}", name=f"yt{ci}"))
        vaug = []
        for t in range(NKT):
            pt = 128 if t < 24 else 64
            va = persist.tile([pt, 130], F32R, tag=f"vaug{t}", name=f"vaug{t}")
            vaug.append(va)
            nc.vector.tensor_copy(va[:, 64:65], ones_col[0:pt, :])
            nc.vector.tensor_copy(va[:, 129:130], ones_col[0:pt, :])

        # ---------------- QKV + V-transpose phase ----------------
        with tc.tile_pool(name="xpool", bufs=3) as xpool, \
             tc.tile_pool(name="qkvps", bufs=3, space="PSUM") as qkvps, \
             tc.tile_pool(name="trps", bufs=2, space="PSUM") as trps, \
             tc.tile_pool(name="vstage", bufs=2) as vstage, \
             nc.named_scope("qkv"):
            for ci, (q0, W) in enumerate(CHUNKS):
                xt = xpool.tile([128, 4 * W], BF16, tag="xt", name=f"xt{ci}")
                for ct in range(4):
                    nc.sync.dma_start(
                        out=xt[:, ct * W:(ct + 1) * W],
                        in_=d['xT'][ct * 128:(ct + 1) * 128, q0:q0 + W])
                for which, wsb in enumerate((wq_sb, wk_sb, wv_sb)):
                    ps = qkvps.tile([128, 512], F32, tag="qkvps",
                                    name=f"ps{ci}_{which}")
                    for ct in range(4):
                        nc.tensor.matmul(
                            ps[:, 0:W],
                            lhsT=wsb[:, ct * 128:(ct + 1) * 128],
                            rhs=xt[:, ct * W:(ct + 1) * W],
                            start=(ct == 0), stop=(ct == 3))
                    if which == 0:
                        nc.vector.tensor_copy(qt_t[ci], ps[:, 0:W])
                    elif which == 1:
                        nc.vector.tensor_copy(kt_t[ci], ps[:, 0:W])
                    else:
                        vs = vstage.tile([128, 512], F32, tag="vs",
                                         name=f"vs{ci}")
                        nc.vector.tensor_copy(vs[:, 0:W], ps[:, 0:W])
                        for i in range((W + 127) // 128):
                            seg = min(128, W - i * 128)
                            t = (q0 + i * 128) // 128
                            tr = trps.tile([128, 128], F32, tag="tr",
                                           name=f"tr{t}")
                            nc.tensor.transpose(tr[0:seg, :],
                                                vs[:, i * 128:i * 128 + seg],
                                                identity)
                            nc.vector.tensor_copy(vaug[t][:, 0:64],
                                                  tr[0:seg, 0:64])
                            nc.vector.tensor_copy(vaug[t][:, 65:129],
                                                  tr[0:seg, 64:128])

        # ---------------- attention + projection ----------------
        stps = es.enter_context(tc.tile_pool(name="stps", bufs=2, space="PSUM"))
        yps = es.enter_context(tc.tile_pool(name="yps", bufs=2, space="PSUM"))
        epool = es.enter_context(tc.tile_pool(name="epool", bufs=4))
        npool = es.enter_context(tc.tile_pool(name="npool", bufs=2))
        rbps = es.enter_context(tc.tile_pool(name="rbps", bufs=2, space="PSUM"))
        projps = es.enter_context(tc.tile_pool(name="projps", bufs=2, space="PSUM"))
        outstage = es.enter_context(tc.tile_pool(name="outstage", bufs=3))

        for ci, (q0, W) in enumerate(CHUNKS):
            pairs = _pairs(ci)
            with nc.named_scope(f"attn{ci}"):
                for h in range(2):
                    hs = slice(h * 64, (h + 1) * 64)
                    yacc = yps.tile([65, 512], F32, tag="yacc",
                                    name=f"yacc{ci}_{h}")
                    for pi, (kt, z, tri) in enumerate(pairs):
                        pt = 128 if kt < 24 else 64
                        w0 = z * 128
                        kci, kof = kt // 4, (kt % 4) * 128
                        st = stps.tile([128, 512], F32, tag="st",
                                       name=f"st{ci}_{h}_{kt}")
                        nc.tensor.matmul(
                            st[0:pt, w0:W],
                            lhsT=kt_t[kci][hs, kof:kof + pt],
                            rhs=qt_t[ci][hs, w0:W],
                            start=True, stop=True)
                        et = epool.tile([128, 512], F32R, tag="et",
                                        name=f"et{ci}_{h}_{kt}")
                        nc.scalar.activation(et[0:pt, w0:W], st[0:pt, w0:W],
                                             AF.Exp)
                        if tri == 'causal':
                            nc.vector.tensor_mul(et[:, w0:w0 + 128],
                                                 et[:, w0:w0 + 128], causal01)
                        elif tri == 'strict':
                            nc.vector.tensor_mul(et[:, w0:w0 + 128],
                                                 et[:, w0:w0 + 128], strict01)
                        elif tri == 'text':
                            m01 = up01_sb if ci in (2, 3) else low01_sb
                            off = q0 - (1024 if ci in (2, 3) else 2048)
                            nc.vector.tensor_mul(et[0:64, 0:W], et[0:64, 0:W],
                                                 m01[:, off:off + W])
                        nc.tensor.matmul(
                            yacc[0:65, w0:W],
                            lhsT=vaug[kt][0:pt, h * 65:h * 65 + 65],
                            rhs=et[0:pt, w0:W],
                            start=(pi == 0), stop=(pi == len(pairs) - 1))
                    # normalize: row 64 of yacc holds the softmax denominators
                    rrow = npool.tile([1, 512], F32R, tag="rrow",
                                      name=f"rr{ci}_{h}")
                    with nc.allow_low_precision(reason="f32r softmax denom"):
                        nc.vector.reciprocal(rrow[:, 0:W], yacc[64:65, 0:W])
                    rb = rbps.tile([64, 512], F32, tag="rb", name=f"rb{ci}_{h}")
                    nc.tensor.matmul(rb[:, 0:W], lhsT=ones1,
                                     rhs=rrow[:, 0:W], start=True,
                                     stop=True)
                    rb_sb = npool.tile([64, 512], F32, tag="rbsb",
                                       name=f"rbsb{ci}_{h}")
                    nc.vector.tensor_copy(rb_sb[:, 0:W], rb[:, 0:W])
                    nc.vector.tensor_mul(yt_t[ci][hs, :], yacc[0:64, 0:W],
                                         rb_sb[:, 0:W])
            with nc.named_scope(f"proj{ci}"):
                for jt in range(4):
                    pps = projps.tile([128, 512], F32, tag="pps",
                                      name=f"pps{ci}_{jt}")
                    nc.tensor.matmul(pps[:, 0:W],
                                     lhsT=wp_sb[:, jt * 128:(jt + 1) * 128],
                                     rhs=yt_t[ci], start=True, stop=True)
                    ob = outstage.tile([128, 512], F32, tag="ob",
                                       name=f"ob{ci}_{jt}")
                    nc.vector.tensor_copy(ob[:, 0:W], pps[:, 0:W])
                    nc.sync.dma_start(
                        out=d['outT'][jt * 128:(jt + 1) * 128, q0:q0 + W],
                        in_=ob[:, 0:W])


_NC_CACHE = None


def _program():
    global _NC_CACHE
    if _NC_CACHE is not None:
        return _NC_CACHE
    nc = bacc.Bacc()
    d = {
        'xT': nc.declare_dram_parameter('xT', [C, L], BF16, isOutput=False).ap(),
        'wqT': nc.declare_dram_parameter('wqT', [C, 128], BF16, isOutput=False).ap(),
        'wkT': nc.declare_dram_parameter('wkT', [C, 128], BF16, isOutput=False).ap(),
        'wvT': nc.declare_dram_parameter('wvT', [C, 128], BF16, isOutput=False).ap(),
        'wpT': nc.declare_dram_parameter('wpT', [128, C], F32, isOutput=False).ap(),
        'up01': nc.declare_dram_parameter('up01', [64, T], F32, isOutput=False).ap(),
        'low01': nc.declare_dram_parameter('low01', [64, T], F32, isOutput=False).ap(),
        'outT': nc.declare_dram_parameter('outT', [C, L], F32, isOutput=True).ap(),
    }
    with tile.TileContext(nc) as tc:
        _emit(nc, tc, d)
    nc.finalize()
    _NC_CACHE = nc
    return nc


def _in_maps(inputs):
    x = np.asarray(inputs['x'], np.float32)
    Wq = np.asarray(inputs['W_q'], np.float32)
    Wk = np.asarray(inputs['W_k'], np.float32)
    Wv = np.asarray(inputs['W_v'], np.float32)
    Wp = np.asarray(inputs['W_proj'], np.float32)
    bq = np.asarray(inputs['b_q'], np.float32)
    bk = np.asarray(inputs['b_k'], np.float32)
    bv = np.asarray(inputs['b_v'], np.float32)
    sf = np.asarray(inputs['start_frames'])
    ef = np.asarray(inputs['end_frames'])

    scale = 1.0 / np.sqrt(HD)
    maps = []
    for core in range(N_CORES):
        b, g = core // 4, core % 4
        sl = slice(g * 128, (g + 1) * 128)
        rs = sf[b] // 8
        re = ef[b] // 8
        f = np.arange(T)
        act = ((f[None, :] >= rs[:, None]) & (f[None, :] < re[:, None])
               ).astype(np.float32)  # [16, T]
        z16 = np.zeros_like(act)
        up01 = np.concatenate([act, z16, act, act], 0)   # [64, T]
        low01 = np.concatenate([z16, act, act, act], 0)
        maps.append({
            'xT': np.ascontiguousarray(x[b].T).astype(ml_dtypes.bfloat16),
            'wqT': np.ascontiguousarray((Wq[sl] * scale).T).astype(ml_dtypes.bfloat16),
            'wkT': np.ascontiguousarray(Wk[sl].T).astype(ml_dtypes.bfloat16),
            'wvT': np.ascontiguousarray(Wv[sl].T).astype(ml_dtypes.bfloat16),
            'wpT': np.ascontiguousarray(Wp[:, sl].T),
            'up01': np.ascontiguousarray(up01),
            'low01': np.ascontiguousarray(low01),
        })
    return maps


def _assemble(results, inputs):
    bp = np.asarray(inputs['b_proj'], np.float32)
    bv = np.asarray(inputs['b_v'], np.float32)
    Wp = np.asarray(inputs['W_proj'], np.float32)
    const = bp + bv @ Wp.T  # b_v passes through softmax-weighted avg exactly
    out = np.empty((B, L, C), np.float32)
    for b in range(B):
        acc = results[b * 4]['outT'].astype(np.float32).copy()
        for g in range(1, 4):
            acc += results[b * 4 + g]['outT']
        out[b] = acc.T + const[None, :]
    return out


def kernel(**inputs):
    nc = _program()
    maps = _in_maps(inputs)
    res = run_bass_kernel_spmd(nc, maps, core_ids=list(range(N_CORES))).results
    return _assemble(res, inputs)


# revision 12
# speedup vs baseline: 7.5384x; 7.5384x over previous
"""Trainium2 Bass kernel: CausalCrossConditionalSelfAttention.

Sharding: 8 cores = (batch b in {0,1}) x (head-group g in {0..3}); each core
computes attention for 2 heads (128 channels) of one batch element, plus its
tensor-parallel slice of the output projection. The host sums the 4 partial
projections per batch and adds b_proj.

On-device layout is transposed (channels on partitions):
  qT/kT: [128 (2 heads x 64 d), L]; S^T chunks [k-tile 128, q 512] so softmax
  reduction happens via a ones-column appended to V in the P^T@V matmul.
Block-causal mask structure is applied as 0/1 multiplicative masks on exp(S),
with fully-masked (k-tile, q-chunk) pairs skipped entirely.
"""

import ml_dtypes
import numpy as np

import concourse.bass as bass
import concourse.mybir as mybir
import concourse.tile as tile
from concourse import bacc
from concourse.bass_utils import run_bass_kernel_spmd
from concourse.masks import make_identity

B = 2
T = 1024
NSEG = 16
C = 512
NH = 8
HD = 64
L = 3 * T + 4 * NSEG  # 3136
N_CORES = 8

F32 = mybir.dt.float32
F32R = mybir.dt.float32r
BF16 = mybir.dt.bfloat16
AF = mybir.ActivationFunctionType
ALU = mybir.AluOpType

CHUNKS = [(0, 512), (512, 512), (1024, 512), (1536, 512), (2048, 512),
          (2560, 512), (3072, 64)]
NKT = 25  # key tiles of 128 (kt 24 has only 64 rows: the 4N text keys)

# Visibility of key-block bb from query-block r, as "keep iff q - k >= D'".
# None = invisible. STRICT marks exclusive (j < i) relations.
DPRIME = [[0, None, None], [1024, 1, -1023], [2048, 1024, 1]]
STRICT = [[False, None, None], [False, True, True], [False, False, True]]


def _pairs(ci):
    """(kt, z, tri) per key-tile for query chunk ci.

    z = number of fully-masked leading 128-subtiles (compute starts at col
    z*128); tri in {None, 'causal', 'strict', 'text'} selects the fixup
    applied to exp(S) for the partially-masked subtile."""
    q0, W = CHUNKS[ci]
    if ci == 6:
        return [(kt, 0, None) for kt in range(NKT)]
    r = q0 // T
    out = []
    for bb in range(3):
        Dp = DPRIME[r][bb]
        if Dp is None:
            continue
        st = STRICT[r][bb]
        D = Dp - 1 if st else Dp
        for kt in range(8 * bb, 8 * bb + 8):
            k0 = kt * 128
            if (q0 + W - 1) - k0 < Dp:
                continue  # fully masked
            if q0 - (k0 + 127) >= Dp:
                out.append((kt, 0, None))  # fully kept
            else:
                o = (k0 + D - q0) // 128
                out.append((kt, o, 'strict' if st else 'causal'))
    if r >= 1:
        out.append((24, 0, 'text'))
    return out


def _emit(nc, tc, d, sfx=''):
    from contextlib import ExitStack

    def fr(ap):
        return ap.bitcast(F32R)

    es = ExitStack()
    with es:
        const = es.enter_context(tc.tile_pool(name="const" + sfx, bufs=1))
        persist = es.enter_context(tc.tile_pool(name="persist" + sfx, bufs=1))

        identity = const.tile([128, 128], F32, tag="ident", name="identity")
        make_identity(nc, identity)
        # Build 0/1 triangular masks in f32, then round-copy into f32r tiles
        # (memset/affine_select cannot write f32r directly).
        causal01f = const.tile([128, 128], F32, tag="causal01f", name="causal01f")
        strict01f = const.tile([128, 128], F32, tag="strict01f", name="strict01f")
        causal01 = const.tile([128, 128], F32R, tag="causal01", name="causal01")
        strict01 = const.tile([128, 128], F32R, tag="strict01", name="strict01")
        for m01f, m01, op in ((causal01f, causal01, ALU.is_ge),
                              (strict01f, strict01, ALU.is_gt)):
            nc.vector.memset(m01f, 1.0)
            # keep (value (-1)*p + 1*f >= / > 0), else fill 0
            nc.gpsimd.affine_select(out=m01f, in_=m01f, pattern=[[1, 128]],
                                    compare_op=op, fill=0.0, base=0,
                                    channel_multiplier=-1)
            nc.vector.tensor_copy(m01, m01f)

        ones_col = const.tile([128, 1], F32, tag="ones_col", name="ones_col")
        nc.vector.memset(ones_col, 1.0)

        wq_sb = const.tile([128, 512], BF16, tag="wq", name="wq_sb")
        wk_sb = const.tile([128, 512], BF16, tag="wk", name="wk_sb")
        wv_sb = const.tile([128, 512], BF16, tag="wv", name="wv_sb")
        for sb, nm in ((wq_sb, 'wqT'), (wk_sb, 'wkT'), (wv_sb, 'wvT')):
            for ct in range(4):
                nc.sync.dma_start(out=sb[:, ct * 128:(ct + 1) * 128],
                                  in_=d[nm][ct * 128:(ct + 1) * 128, :])
        wp_stage = const.tile([128, 512], F32, tag="wps", name="wp_stage")
        nc.sync.dma_start(out=wp_stage, in_=d['wpT'][:, :])
        wp_sb = const.tile([128, 512], F32R, tag="wp", name="wp_sb")
        nc.vector.tensor_copy(wp_sb, wp_stage)
        up01_st = const.tile([64, T], F32, tag="up01st", name="up01_st")
        low01_st = const.tile([64, T], F32, tag="low01st", name="low01_st")
        nc.sync.dma_start(out=up01_st, in_=d['up01'][:, :])
        nc.sync.dma_start(out=low01_st, in_=d['low01'][:, :])
        up01_sb = const.tile([64, T], F32R, tag="up01", name="up01_sb")
        low01_sb = const.tile([64, T], F32R, tag="low01", name="low01_sb")
        nc.vector.tensor_copy(up01_sb, up01_st)
        nc.vector.tensor_copy(low01_sb, low01_st)

        # Persistent per-chunk tensors
        qt_t, kt_t, yt_t = [], [], []
        for ci, (q0, W) in enumerate(CHUNKS):
            qt_t.append(persist.tile([128, W], F32R, tag=f"qt{ci}", name=f"qt{ci}"))
            kt_t.append(persist.tile([128, W], F32R, tag=f"kt{ci}", name=f"kt{ci}"))
            yt_t.append(persist.tile([128, W], F32R, tag=f"yt{ci}", name=f"yt{ci}"))
        vaug = []
        for t in range(NKT):
            pt = 128 if t < 24 else 64
            va = persist.tile([pt, 130], F32R, tag=f"vaug{t}", name=f"vaug{t}")
            vaug.append(va)
            nc.vector.tensor_copy(va[:, 64:65], ones_col[0:pt, :])
            nc.vector.tensor_copy(va[:, 129:130], ones_col[0:pt, :])

        # ---------------- QKV + V-transpose phase ----------------
        with tc.tile_pool(name="xpool" + sfx, bufs=3) as xpool, \
             tc.tile_pool(name="qkvps" + sfx, bufs=3, space="PSUM") as qkvps, \
             tc.tile_pool(name="trps" + sfx, bufs=2, space="PSUM") as trps, \
             tc.tile_pool(name="vstage" + sfx, bufs=2) as vstage, \
             nc.named_scope("qkv" + sfx):
            for ci, (q0, W) in enumerate(CHUNKS):
                xt = xpool.tile([128, 4 * W], BF16, tag="xt", name=f"xt{ci}")
                for ct in range(4):
                    nc.sync.dma_start(
                        out=xt[:, ct * W:(ct + 1) * W],
                        in_=d['xT'][ct * 128:(ct + 1) * 128, q0:q0 + W])
                for which, wsb in enumerate((wq_sb, wk_sb, wv_sb)):
                    ps = qkvps.tile([128, 512], F32, tag="qkvps",
                                    name=f"ps{ci}_{which}")
                    for ct in range(4):
                        nc.tensor.matmul(
                            ps[:, 0:W],
                            lhsT=wsb[:, ct * 128:(ct + 1) * 128],
                            rhs=xt[:, ct * W:(ct + 1) * W],
                            start=(ct == 0), stop=(ct == 3))
                    if which == 0:
                        nc.scalar.activation(qt_t[ci], ps[:, 0:W], AF.Copy)
                    elif which == 1:
                        nc.scalar.activation(kt_t[ci], ps[:, 0:W], AF.Copy)
                    else:
                        vs = vstage.tile([128, 512], F32, tag="vs",
                                         name=f"vs{ci}")
                        nc.vector.tensor_copy(vs[:, 0:W], ps[:, 0:W])
                        for i in range((W + 127) // 128):
                            seg = min(128, W - i * 128)
                            t = (q0 + i * 128) // 128
                            tr = trps.tile([128, 128], F32, tag="tr",
                                           name=f"tr{t}")
                            nc.tensor.transpose(tr[0:seg, :],
                                                vs[:, i * 128:i * 128 + seg],
                                                identity)
                            nc.vector.tensor_copy(vaug[t][:, 0:64],
                                                  tr[0:seg, 0:64])
                            nc.vector.tensor_copy(vaug[t][:, 65:129],
                                                  tr[0:seg, 64:128])

        # ---------------- attention + projection ----------------
        stps = es.enter_context(tc.tile_pool(name="stps" + sfx, bufs=3, space="PSUM"))
        yps = es.enter_context(tc.tile_pool(name="yps" + sfx, bufs=2, space="PSUM"))
        epool = es.enter_context(tc.tile_pool(name="epool" + sfx, bufs=6))
        npool = es.enter_context(tc.tile_pool(name="npool" + sfx, bufs=2))
        projps = es.enter_context(tc.tile_pool(name="projps" + sfx, bufs=2, space="PSUM"))
        outstage = es.enter_context(tc.tile_pool(name="outstage" + sfx, bufs=3))

        for ci, (q0, W) in enumerate(CHUNKS):
            pairs = _pairs(ci)
            with nc.named_scope(f"attn{ci}" + sfx):
                for h in range(2):
                    hs = slice(h * 64, (h + 1) * 64)
                    yacc = yps.tile([65, 512], F32, tag="yacc",
                                    name=f"yacc{ci}_{h}")
                    for pi, (kt, z, tri) in enumerate(pairs):
                        pt = 128 if kt < 24 else 64
                        w0 = z * 128
                        kci, kof = kt // 4, (kt % 4) * 128
                        st = stps.tile([128, 512], F32, tag="st",
                                       name=f"st{ci}_{h}_{kt}")
                        nc.tensor.matmul(
                            st[0:pt, w0:W],
                            lhsT=kt_t[kci][hs, kof:kof + pt],
                            rhs=qt_t[ci][hs, w0:W],
                            start=True, stop=True)
                        et = epool.tile([128, 512], F32R, tag="et",
                                        name=f"et{ci}_{h}_{kt}")
                        nc.scalar.activation(et[0:pt, w0:W], st[0:pt, w0:W],
                                             AF.Exp)
                        if tri == 'causal':
                            nc.vector.tensor_mul(et[:, w0:w0 + 128],
                                                 et[:, w0:w0 + 128], causal01)
                        elif tri == 'strict':
                            nc.vector.tensor_mul(et[:, w0:w0 + 128],
                                                 et[:, w0:w0 + 128], strict01)
                        elif tri == 'text':
                            m01 = up01_sb if ci in (2, 3) else low01_sb
                            off = q0 - (1024 if ci in (2, 3) else 2048)
                            nc.vector.tensor_mul(et[0:64, 0:W], et[0:64, 0:W],
                                                 m01[:, off:off + W])
                        nc.tensor.matmul(
                            yacc[0:65, w0:W],
                            lhsT=vaug[kt][0:pt, h * 65:h * 65 + 65],
                            rhs=et[0:pt, w0:W],
                            start=(pi == 0), stop=(pi == len(pairs) - 1))
                    # normalize: row 64 of yacc holds the softmax denominators
                    rrow = npool.tile([1, 512], F32, tag="rrow",
                                      name=f"rr{ci}_{h}")
                    nc.vector.reciprocal(rrow[:, 0:W], yacc[64:65, 0:W])
                    rb_sb = npool.tile([64, 512], F32, tag="rbsb",
                                       name=f"rbsb{ci}_{h}")
                    nc.gpsimd.partition_broadcast(rb_sb[:, 0:W], rrow[:, 0:W])
                    nc.vector.tensor_mul(yt_t[ci][hs, :], yacc[0:64, 0:W],
                                         rb_sb[:, 0:W])
            with nc.named_scope(f"proj{ci}" + sfx):
                for jt in range(4):
                    pps = projps.tile([128, 512], F32, tag="pps",
                                      name=f"pps{ci}_{jt}")
                    nc.tensor.matmul(pps[:, 0:W],
                                     lhsT=wp_sb[:, jt * 128:(jt + 1) * 128],
                                     rhs=yt_t[ci], start=True, stop=True)
                    ob = outstage.tile([128, 512], F32, tag="ob",
                                       name=f"ob{ci}_{jt}")
                    nc.vector.tensor_copy(ob[:, 0:W], pps[:, 0:W])
                    nc.sync.dma_start(
                        out=d['outT'][jt * 128:(jt + 1) * 128, q0:q0 + W],
                        in_=ob[:, 0:W])


_NC_CACHE = None


def _program(passes=1):
    global _NC_CACHE
    if passes == 1 and _NC_CACHE is not None:
        return _NC_CACHE
    nc = bacc.Bacc()
    d = {
        'xT': nc.declare_dram_parameter('xT', [C, L], BF16, isOutput=False).ap(),
        'wqT': nc.declare_dram_parameter('wqT', [C, 128], BF16, isOutput=False).ap(),
        'wkT': nc.declare_dram_parameter('wkT', [C, 128], BF16, isOutput=False).ap(),
        'wvT': nc.declare_dram_parameter('wvT', [C, 128], BF16, isOutput=False).ap(),
        'wpT': nc.declare_dram_parameter('wpT', [128, C], F32, isOutput=False).ap(),
        'up01': nc.declare_dram_parameter('up01', [64, T], F32, isOutput=False).ap(),
        'low01': nc.declare_dram_parameter('low01', [64, T], F32, isOutput=False).ap(),
        'outT': nc.declare_dram_parameter('outT', [C, L], F32, isOutput=True).ap(),
    }
    with tile.TileContext(nc) as tc:
        for p in range(passes):
            _emit(nc, tc, d, sfx=f"_p{p}" if p else "")
    nc.finalize()
    if passes == 1:
        _NC_CACHE = nc
    return nc


def _in_maps(inputs):
    x = np.asarray(inputs['x'], np.float32)
    Wq = np.asarray(inputs['W_q'], np.float32)
    Wk = np.asarray(inputs['W_k'], np.float32)
    Wv = np.asarray(inputs['W_v'], np.float32)
    Wp = np.asarray(inputs['W_proj'], np.float32)
    bq = np.asarray(inputs['b_q'], np.float32)
    bk = np.asarray(inputs['b_k'], np.float32)
    bv = np.asarray(inputs['b_v'], np.float32)
    sf = np.asarray(inputs['start_frames'])
    ef = np.asarray(inputs['end_frames'])

    scale = 1.0 / np.sqrt(HD)
    maps = []
    for core in range(N_CORES):
        b, g = core // 4, core % 4
        sl = slice(g * 128, (g + 1) * 128)
        rs = sf[b] // 8
        re = ef[b] // 8
        f = np.arange(T)
        act = ((f[None, :] >= rs[:, None]) & (f[None, :] < re[:, None])
               ).astype(np.float32)  # [16, T]
        z16 = np.zeros_like(act)
        up01 = np.concatenate([act, z16, act, act], 0)   # [64, T]
        low01 = np.concatenate([z16, act, act, act], 0)
        maps.append({
            'xT': np.ascontiguousarray(x[b].T).astype(ml_dtypes.bfloat16),
            'wqT': np.ascontiguousarray((Wq[sl] * scale).T).astype(ml_dtypes.bfloat16),
            'wkT': np.ascontiguousarray(Wk[sl].T).astype(ml_dtypes.bfloat16),
            'wvT': np.ascontiguousarray(Wv[sl].T).astype(ml_dtypes.bfloat16),
            'wpT': np.ascontiguousarray(Wp[:, sl].T),
            'up01': np.ascontiguousarray(up01),
            'low01': np.ascontiguousarray(low01),
        })
    return maps


def _assemble(results, inputs):
    bp = np.asarray(inputs['b_proj'], np.float32)
    bv = np.asarray(inputs['b_v'], np.float32)
    Wp = np.asarray(inputs['W_proj'], np.float32)
    const = bp + bv @ Wp.T  # b_v passes through softmax-weighted avg exactly
    out = np.empty((B, L, C), np.float32)
    for b in range(B):
        acc = results[b * 4]['outT'].astype(np.float32).copy()
        for g in range(1, 4):
            acc += results[b * 4 + g]['outT']
        out[b] = acc.T + const[None, :]
    return out


def kernel(**inputs):
    nc = _program()
    maps = _in_maps(inputs)
    res = run_bass_kernel_spmd(nc, maps, core_ids=list(range(N_CORES))).results
    return _assemble(res, inputs)


# revision 21
# speedup vs baseline: 14.1473x; 1.8767x over previous
"""Trainium2 Bass kernel: CausalCrossConditionalSelfAttention.

Sharding: 8 cores = (batch b in {0,1}) x (head-group g in {0..3}); each core
computes attention for 2 heads (128 channels) of one batch element, plus its
tensor-parallel slice of the output projection. The host sums the 4 partial
projections per batch and adds b_proj.

On-device layout is transposed (channels on partitions):
  qT/kT: [128 (2 heads x 64 d), L]; S^T chunks [k-tile 128, q 512] so softmax
  reduction happens via a ones-column appended to V in the P^T@V matmul.
Block-causal mask structure is applied as 0/1 multiplicative masks on exp(S),
with fully-masked (k-tile, q-chunk) pairs skipped entirely.
"""

import ml_dtypes
import numpy as np

import concourse.bass as bass
import concourse.mybir as mybir
import concourse.tile as tile
from concourse import bacc
from concourse.bass_utils import run_bass_kernel_spmd
from concourse.masks import make_identity

B = 2
T = 1024
NSEG = 16
C = 512
NH = 8
HD = 64
L = 3 * T + 4 * NSEG  # 3136
N_CORES = 8

F32 = mybir.dt.float32
F32R = mybir.dt.float32r
BF16 = mybir.dt.bfloat16
AF = mybir.ActivationFunctionType
ALU = mybir.AluOpType

CHUNKS = [(0, 512), (512, 512), (1024, 512), (1536, 512), (2048, 512),
          (2560, 512), (3072, 64)]
NKT = 25  # key tiles of 128 (kt 24 has only 64 rows: the 4N text keys)

# Visibility of key-block bb from query-block r, as "keep iff q - k >= D'".
# None = invisible. STRICT marks exclusive (j < i) relations.
DPRIME = [[0, None, None], [1024, 1, -1023], [2048, 1024, 1]]
STRICT = [[False, None, None], [False, True, True], [False, False, True]]


def _pairs(ci):
    """(kt, z, tri) per key-tile for query chunk ci.

    z = number of fully-masked leading 128-subtiles (compute starts at col
    z*128); tri in {None, 'causal', 'strict', 'text'} selects the fixup
    applied to exp(S) for the partially-masked subtile."""
    q0, W = CHUNKS[ci]
    if ci == 6:
        return [(kt, 0, None) for kt in range(NKT)]
    r = q0 // T
    out = []
    for bb in range(3):
        Dp = DPRIME[r][bb]
        if Dp is None:
            continue
        st = STRICT[r][bb]
        D = Dp - 1 if st else Dp
        for kt in range(8 * bb, 8 * bb + 8):
            k0 = kt * 128
            if (q0 + W - 1) - k0 < Dp:
                continue  # fully masked
            if q0 - (k0 + 127) >= Dp:
                out.append((kt, 0, None))  # fully kept
            else:
                o = (k0 + D - q0) // 128
                out.append((kt, o, 'strict' if st else 'causal'))
    if r >= 1:
        out.append((24, 0, 'text'))
    return out


def _emit(nc, tc, d, sfx=''):
    from contextlib import ExitStack

    def fr(ap):
        return ap.bitcast(F32R)

    es = ExitStack()
    with es:
        const = es.enter_context(tc.tile_pool(name="const" + sfx, bufs=1))
        persist = es.enter_context(tc.tile_pool(name="persist" + sfx, bufs=1))

        identity = const.tile([128, 128], F32, tag="ident", name="identity")
        make_identity(nc, identity)
        # Build 0/1 triangular masks in f32, then round-copy into f32r tiles
        # (memset/affine_select cannot write f32r directly).
        causal01f = const.tile([128, 128], F32, tag="causal01f", name="causal01f")
        strict01f = const.tile([128, 128], F32, tag="strict01f", name="strict01f")
        causal01 = const.tile([128, 128], F32R, tag="causal01", name="causal01")
        strict01 = const.tile([128, 128], F32R, tag="strict01", name="strict01")
        for m01f, m01, op in ((causal01f, causal01, ALU.is_ge),
                              (strict01f, strict01, ALU.is_gt)):
            nc.vector.memset(m01f, 1.0)
            # keep (value (-1)*p + 1*f >= / > 0), else fill 0
            nc.gpsimd.affine_select(out=m01f, in_=m01f, pattern=[[1, 128]],
                                    compare_op=op, fill=0.0, base=0,
                                    channel_multiplier=-1)
            nc.vector.tensor_copy(m01, m01f)

        ones_col = const.tile([128, 1], F32, tag="ones_col", name="ones_col")
        nc.vector.memset(ones_col, 1.0)

        wq_sb = const.tile([128, 512], BF16, tag="wq", name="wq_sb")
        wk_sb = const.tile([128, 512], BF16, tag="wk", name="wk_sb")
        wv_sb = const.tile([128, 512], BF16, tag="wv", name="wv_sb")
        for sb, nm in ((wq_sb, 'wqT'), (wk_sb, 'wkT'), (wv_sb, 'wvT')):
            for ct in range(4):
                nc.sync.dma_start(out=sb[:, ct * 128:(ct + 1) * 128],
                                  in_=d[nm][ct * 128:(ct + 1) * 128, :])
        wp_stage = const.tile([128, 512], F32, tag="wps", name="wp_stage")
        nc.sync.dma_start(out=wp_stage, in_=d['wpT'][:, :])
        wp_sb = const.tile([128, 512], F32R, tag="wp", name="wp_sb")
        nc.vector.tensor_copy(wp_sb, wp_stage)
        up01_st = const.tile([64, T], F32, tag="up01st", name="up01_st")
        low01_st = const.tile([64, T], F32, tag="low01st", name="low01_st")
        nc.sync.dma_start(out=up01_st, in_=d['up01'][:, :])
        nc.sync.dma_start(out=low01_st, in_=d['low01'][:, :])
        up01_sb = const.tile([64, T], F32R, tag="up01", name="up01_sb")
        low01_sb = const.tile([64, T], F32R, tag="low01", name="low01_sb")
        nc.vector.tensor_copy(up01_sb, up01_st)
        nc.vector.tensor_copy(low01_sb, low01_st)

        # Persistent per-chunk tensors
        qt_t, kt_t, yt_t = [], [], []
        for ci, (q0, W) in enumerate(CHUNKS):
            qt_t.append(persist.tile([128, W], F32R, tag=f"qt{ci}", name=f"qt{ci}"))
            kt_t.append(persist.tile([128, W], F32R, tag=f"kt{ci}", name=f"kt{ci}"))
            yt_t.append(persist.tile([128, W], F32R, tag=f"yt{ci}", name=f"yt{ci}"))
        vaug = []
        for t in range(NKT):
            pt = 128 if t < 24 else 64
            va = persist.tile([pt, 130], F32R, tag=f"vaug{t}", name=f"vaug{t}")
            vaug.append(va)
            nc.vector.tensor_copy(va[:, 64:65], ones_col[0:pt, :])
            nc.vector.tensor_copy(va[:, 129:130], ones_col[0:pt, :])

        # ---------------- interleaved QKV / attention / proj ----------------
        # One shared PSUM pool; per-tag bufs: mm512 x5 (qkv-accum, S^T, proj)
        # + tr x1 + yacc x2 = 8 banks.
        # PSUM budget (8 banks): st 2x[128,1024] = 4, qkv 1, tr/proj shared 1,
        # yacc 2.
        qkvps = es.enter_context(tc.tile_pool(name="qkvps" + sfx, bufs=1,
                                              space="PSUM"))
        trps = es.enter_context(tc.tile_pool(name="trps" + sfx, bufs=1,
                                             space="PSUM"))
        stps = es.enter_context(tc.tile_pool(name="stps" + sfx, bufs=2,
                                             space="PSUM"))
        yps = es.enter_context(tc.tile_pool(name="yps" + sfx, bufs=2,
                                            space="PSUM"))
        xpool = es.enter_context(tc.tile_pool(name="xpool" + sfx, bufs=7))
        vstage = es.enter_context(tc.tile_pool(name="vstage" + sfx, bufs=2))
        epool = es.enter_context(tc.tile_pool(name="epool" + sfx, bufs=6))
        npool = es.enter_context(tc.tile_pool(name="npool" + sfx, bufs=2))
        outstage = es.enter_context(tc.tile_pool(name="outstage" + sfx, bufs=3))

        def emit_qkv(ci):
            q0, W = CHUNKS[ci]
            with nc.named_scope(f"qkv{ci}" + sfx):
                xt = xpool.tile([128, 4 * 512], BF16, tag="xt", name=f"xt{ci}")
                for ct in range(4):
                    nc.sync.dma_start(
                        out=xt[:, ct * W:(ct + 1) * W],
                        in_=d['xT'][ct * 128:(ct + 1) * 128, q0:q0 + W])
                for which, wsb in enumerate((wq_sb, wk_sb, wv_sb)):
                    mm = qkvps.tile([128, 512], F32, tag="qkvmm",
                                    name=f"ps{ci}_{which}")
                    for ct in range(4):
                        nc.tensor.matmul(
                            mm[:, 0:W],
                            lhsT=wsb[:, ct * 128:(ct + 1) * 128],
                            rhs=xt[:, ct * W:(ct + 1) * W],
                            start=(ct == 0), stop=(ct == 3))
                    if which == 0:
                        nc.scalar.activation(qt_t[ci], mm[:, 0:W], AF.Copy)
                    elif which == 1:
                        nc.vector.tensor_copy(kt_t[ci], mm[:, 0:W])
                    else:
                        vs = vstage.tile([128, 512], F32, tag="vs",
                                         name=f"vs{ci}")
                        nc.vector.tensor_copy(vs[:, 0:W], mm[:, 0:W])
                        for i in range((W + 127) // 128):
                            seg = min(128, W - i * 128)
                            t = (q0 + i * 128) // 128
                            tr = trps.tile([128, 128], F32, tag="tr",
                                           name=f"tr{t}")
                            nc.tensor.transpose(tr[0:seg, :],
                                                vs[:, i * 128:i * 128 + seg],
                                                identity)
                            nc.scalar.activation(vaug[t][:, 0:64],
                                                  tr[0:seg, 0:64], AF.Copy)
                            nc.scalar.activation(vaug[t][:, 65:129],
                                                  tr[0:seg, 64:128], AF.Copy)

        def pack_groups(pairs, W):
            """Pack motion pairs' suffix widths contiguously into [128,1024]
            st tiles. A member may not cross a 512 PSUM-bank boundary; close
            the tile when it would (keeps the exp range hole-free). Text
            pairs (64 valid partitions) go in their own group."""
            groups = []  # list of (members, total) ; member=(kt,z,tri,boff,wdt)
            cur, off = [], 0
            for (kt, z, tri) in pairs:
                if kt == 24:
                    if cur:
                        groups.append((cur, off))
                        cur, off = [], 0
                    groups.append(([(kt, z, tri, 0, W - z * 128)], W - z * 128))
                    continue
                wdt = W - z * 128
                if off + wdt > 1024 or (off // 512 != (off + wdt - 1) // 512):
                    groups.append((cur, off))
                    cur, off = [], 0
                cur.append((kt, z, tri, off, wdt))
                off += wdt
            if cur:
                groups.append((cur, off))
            return groups

        def emit_attn(ci):
            q0, W = CHUNKS[ci]
            pairs = _pairs(ci)
            groups = pack_groups(pairs, W)
            npairs = len(pairs)
            with nc.named_scope(f"attn{ci}" + sfx):
                for h in range(2):
                    hs = slice(h * 64, (h + 1) * 64)
                    yacc = yps.tile([65, 512], F32, tag="yacc",
                                    name=f"yacc{ci}_{h}")
                    pi = 0
                    for gi, (members, gw) in enumerate(groups):
                        gpt = 64 if members[0][0] == 24 else 128
                        st = stps.tile([128, 1024], F32, tag="st",
                                       name=f"st{ci}_{h}_{gi}")
                        for (kt, z, tri, boff, wdt) in members:
                            pt = 128 if kt < 24 else 64
                            w0 = z * 128
                            kci, kof = kt // 4, (kt % 4) * 128
                            nc.tensor.matmul(
                                st[0:pt, boff:boff + wdt],
                                lhsT=kt_t[kci][hs, kof:kof + pt],
                                rhs=qt_t[ci][hs, w0:W],
                                start=True, stop=True)
                        et = epool.tile([128, 1024], F32R, tag="et",
                                        name=f"et{ci}_{h}_{gi}")
                        nc.scalar.activation(et[0:gpt, 0:gw], st[0:gpt, 0:gw],
                                             AF.Exp)
                        for (kt, z, tri, boff, wdt) in members:
                            pt = 128 if kt < 24 else 64
                            w0 = z * 128
                            if tri == 'causal':
                                nc.vector.tensor_mul(et[:, boff:boff + 128],
                                                     et[:, boff:boff + 128],
                                                     causal01)
                            elif tri == 'strict':
                                nc.vector.tensor_mul(et[:, boff:boff + 128],
                                                     et[:, boff:boff + 128],
                                                     strict01)
                            elif tri == 'text':
                                m01 = up01_sb if ci in (2, 3) else low01_sb
                                off = q0 - (1024 if ci in (2, 3) else 2048)
                                nc.vector.tensor_mul(
                                    et[0:64, boff:boff + wdt],
                                    et[0:64, boff:boff + wdt],
                                    m01[:, off + w0:off + W])
                            nc.tensor.matmul(
                                yacc[0:65, w0:W],
                                lhsT=vaug[kt][0:pt, h * 65:h * 65 + 65],
                                rhs=et[0:pt, boff:boff + wdt],
                                start=(pi == 0), stop=(pi == npairs - 1))
                            pi += 1
                    rrow = npool.tile([1, 512], F32, tag="rrow",
                                      name=f"rr{ci}_{h}")
                    nc.vector.reciprocal(rrow[:, 0:W], yacc[64:65, 0:W])
                    rb_sb = npool.tile([64, 512], F32, tag="rbsb",
                                       name=f"rbsb{ci}_{h}")
                    nc.gpsimd.partition_broadcast(rb_sb[:, 0:W], rrow[:, 0:W])
                    nc.vector.tensor_mul(yt_t[ci][hs, :], yacc[0:64, 0:W],
                                         rb_sb[:, 0:W])

        def emit_proj(ci):
            q0, W = CHUNKS[ci]
            with nc.named_scope(f"proj{ci}" + sfx):
                for jt in range(4):
                    pps = trps.tile([128, 512], F32, tag="tr",
                                    name=f"pps{ci}_{jt}")
                    nc.tensor.matmul(pps[:, 0:W],
                                     lhsT=wp_sb[:, jt * 128:(jt + 1) * 128],
                                     rhs=yt_t[ci], start=True, stop=True)
                    ob = outstage.tile([128, 512], F32, tag="ob",
                                       name=f"ob{ci}_{jt}")
                    nc.vector.tensor_copy(ob[:, 0:W], pps[:, 0:W])
                    nc.sync.dma_start(
                        out=d['outT'][jt * 128:(jt + 1) * 128, q0:q0 + W],
                        in_=ob[:, 0:W])

        # Interleave QKV and attention respecting key-chunk needs:
        # attn0 needs kt chunk {0}; attn1 {0,1}; attn2 {0,2,4,6} (r1 sees
        # block-2 keys); attn3+ need all. Text keys (chunk 6) come first.
        emit_qkv(6)
        emit_qkv(0)
        emit_attn(0)
        emit_proj(0)
        emit_qkv(1)
        emit_attn(1)
        emit_proj(1)
        emit_qkv(2)
        emit_qkv(4)
        emit_attn(2)
        emit_proj(2)
        emit_attn(4)
        emit_proj(4)
        emit_qkv(3)
        emit_qkv(5)
        for ci in (3, 5, 6):
            emit_attn(ci)
            emit_proj(ci)



_NC_CACHE = None


def _program(passes=1):
    global _NC_CACHE
    if passes == 1 and _NC_CACHE is not None:
        return _NC_CACHE
    nc = bacc.Bacc()
    d = {
        'xT': nc.declare_dram_parameter('xT', [C, L], BF16, isOutput=False).ap(),
        'wqT': nc.declare_dram_parameter('wqT', [C, 128], BF16, isOutput=False).ap(),
        'wkT': nc.declare_dram_parameter('wkT', [C, 128], BF16, isOutput=False).ap(),
        'wvT': nc.declare_dram_parameter('wvT', [C, 128], BF16, isOutput=False).ap(),
        'wpT': nc.declare_dram_parameter('wpT', [128, C], F32, isOutput=False).ap(),
        'up01': nc.declare_dram_parameter('up01', [64, T], F32, isOutput=False).ap(),
        'low01': nc.declare_dram_parameter('low01', [64, T], F32, isOutput=False).ap(),
        'outT': nc.declare_dram_parameter('outT', [C, L], F32, isOutput=True).ap(),
    }
    with tile.TileContext(nc) as tc:
        for p in range(passes):
            _emit(nc, tc, d, sfx=f"_p{p}" if p else "")
    nc.finalize()
    if passes == 1:
        _NC_CACHE = nc
    return nc


def _in_maps(inputs):
    x = np.asarray(inputs['x'], np.float32)
    Wq = np.asarray(inputs['W_q'], np.float32)
    Wk = np.asarray(inputs['W_k'], np.float32)
    Wv = np.asarray(inputs['W_v'], np.float32)
    Wp = np.asarray(inputs['W_proj'], np.float32)
    bq = np.asarray(inputs['b_q'], np.float32)
    bk = np.asarray(inputs['b_k'], np.float32)
    bv = np.asarray(inputs['b_v'], np.float32)
    sf = np.asarray(inputs['start_frames'])
    ef = np.asarray(inputs['end_frames'])

    scale = 1.0 / np.sqrt(HD)
    maps = []
    for core in range(N_CORES):
        b, g = core // 4, core % 4
        sl = slice(g * 128, (g + 1) * 128)
        rs = sf[b] // 8
        re = ef[b] // 8
        f = np.arange(T)
        act = ((f[None, :] >= rs[:, None]) & (f[None, :] < re[:, None])
               ).astype(np.float32)  # [16, T]
        z16 = np.zeros_like(act)
        up01 = np.concatenate([act, z16, act, act], 0)   # [64, T]
        low01 = np.concatenate([z16, act, act, act], 0)
        maps.append({
            'xT': np.ascontiguousarray(x[b].T).astype(ml_dtypes.bfloat16),
            'wqT': np.ascontiguousarray((Wq[sl] * scale).T).astype(ml_dtypes.bfloat16),
            'wkT': np.ascontiguousarray(Wk[sl].T).astype(ml_dtypes.bfloat16),
            'wvT': np.ascontiguousarray(Wv[sl].T).astype(ml_dtypes.bfloat16),
            'wpT': np.ascontiguousarray(Wp[:, sl].T),
            'up01': np.ascontiguousarray(up01),
            'low01': np.ascontiguousarray(low01),
        })
    return maps


def _assemble(results, inputs):
    bp = np.asarray(inputs['b_proj'], np.float32)
    bv = np.asarray(inputs['b_v'], np.float32)
    Wp = np.asarray(inputs['W_proj'], np.float32)
    const = bp + bv @ Wp.T  # b_v passes through softmax-weighted avg exactly
    out = np.empty((B, L, C), np.float32)
    for b in range(B):
        acc = results[b * 4]['outT'].astype(np.float32).copy()
        for g in range(1, 4):
            acc += results[b * 4 + g]['outT']
        out[b] = acc.T + const[None, :]
    return out


def kernel(**inputs):
    nc = _program()
    maps = _in_maps(inputs)
    res = run_bass_kernel_spmd(nc, maps, core_ids=list(range(N_CORES))).results
    return _assemble(res, inputs)
